# revision 6
# baseline (speedup 1.0000x reference)
"""Distribution cross-entropy loss on 8 Trainium2 NeuronCores.

loss = -(1/B) * sum(preds_t * log(preds_s)),  preds_* : [4096, 1000] f32

Data-parallel: batch dim sharded 8 ways (512 rows/core). Each core streams
its 2x2MB shard, computes log on the ACT engine, fused multiply+row-reduce
on the DVE, and writes a [128, NT] per-partition partial-sum tile. The final
(tiny) reduction over 8*128*NT values happens on the host in float64.
"""

import numpy as np

import concourse.bacc as bacc
import concourse.bass as bass
import concourse.tile as tile
from concourse import mybir
from concourse.bass_utils import run_bass_kernel_spmd

N_CORES = 8
B, C = 4096, 1000
ROWS = B // N_CORES  # 512 rows per core
P = 128              # SBUF partitions
NT = ROWS // P       # 4 tiles per core

_NC_CACHE = {}


def _build_nc():
    if "nc" in _NC_CACHE:
        return _NC_CACHE["nc"]
    nc = bacc.Bacc("TRN2", debug=False)
    s_ap = nc.dram_tensor("preds_s", [ROWS, C], mybir.dt.float32, kind="ExternalInput").ap()
    t_ap = nc.dram_tensor("preds_t", [ROWS, C], mybir.dt.float32, kind="ExternalInput").ap()
    out_ap = nc.dram_tensor("partial", [P, NT], mybir.dt.float32, kind="ExternalOutput").ap()

    s3 = s_ap.rearrange("(n p) c -> n p c", p=P)
    t3 = t_ap.rearrange("(n p) c -> n p c", p=P)

    with tile.TileContext(nc) as tc:
        with (
            tc.tile_pool(name="io", bufs=2) as io_pool,
            tc.tile_pool(name="work", bufs=2) as work_pool,
            tc.tile_pool(name="acc", bufs=1) as acc_pool,
        ):
            acc = acc_pool.tile([P, NT], mybir.dt.float32)
            for i in range(NT):
                s_tile = io_pool.tile([P, C], mybir.dt.float32, tag="s")
                nc.sync.dma_start(out=s_tile[:], in_=s3[i])
                t_tile = io_pool.tile([P, C], mybir.dt.float32, tag="t")
                nc.sync.dma_start(out=t_tile[:], in_=t3[i])

                log_tile = work_pool.tile([P, C], mybir.dt.float32, tag="log")
                nc.scalar.activation(
                    out=log_tile[:],
                    in_=s_tile[:],
                    func=mybir.ActivationFunctionType.Ln,
                )
                prod = work_pool.tile([P, C], mybir.dt.float32, tag="prod")
                nc.vector.tensor_mul(prod[:], log_tile[:], t_tile[:])
                nc.vector.tensor_reduce(
                    out=acc[:, i : i + 1],
                    in_=prod[:],
                    axis=mybir.AxisListType.X,
                    op=mybir.AluOpType.add,
                )
            nc.sync.dma_start(out=out_ap, in_=acc[:])
    nc.compile()
    _NC_CACHE["nc"] = nc
    return nc


def kernel(preds_s, preds_t):
    preds_s = np.ascontiguousarray(np.asarray(preds_s, dtype=np.float32))
    preds_t = np.ascontiguousarray(np.asarray(preds_t, dtype=np.float32))
    assert preds_s.shape == (B, C) and preds_t.shape == (B, C)

    nc = _build_nc()
    rs = preds_s.reshape(N_CORES, ROWS, C)
    rt = preds_t.reshape(N_CORES, ROWS, C)
    in_maps = [
        {"preds_s": np.ascontiguousarray(rs[k]), "preds_t": np.ascontiguousarray(rt[k])}
        for k in range(N_CORES)
    ]
    res = run_bass_kernel_spmd(nc, in_maps, core_ids=list(range(N_CORES)))
    total = 0.0
    for r in res.results:
        total += r["partial"].astype(np.float64).sum()
    return np.asarray(-total / B, dtype=np.float32)


# revision 8
# speedup vs baseline: 1.0699x; 1.0699x over previous
"""Distribution cross-entropy loss on 8 Trainium2 NeuronCores.

loss = -(1/B) * sum(preds_t * log(preds_s)),  preds_* : [4096, 1000] f32

Data-parallel: batch dim sharded 8 ways (512 rows/core). Each core streams
its 2x2MB shard (s tiles via the Sync HWDGE ring, t tiles via the GpSimd
SWDGE ring so descriptor generation is parallel), computes log on the ACT
engine, and a fused multiply+row-sum on the DVE (scalar_tensor_tensor with
a stride-0 dummy main output). Raw Bacc with manual semaphores - no
TileContext, so no tile entry/exit all-engine barriers. Per-core output is
a [128, NT] partial-sum tile; the final tiny reduction over 8*128*NT values
happens on the host in float64.
"""

import numpy as np

import concourse.bacc as bacc
import concourse.bass as bass
from concourse import mybir
from concourse.bass_utils import run_bass_kernel_spmd

N_CORES = 8
B, C = 4096, 1000
ROWS = B // N_CORES  # 512 rows per core
P = 128              # SBUF partitions
NT = ROWS // P       # 4 tiles per core

_NC_CACHE = {}


def _build_nc():
    if "nc" in _NC_CACHE:
        return _NC_CACHE["nc"]
    nc = bacc.Bacc("TRN2", debug=False)
    s_ap = nc.dram_tensor("preds_s", [ROWS, C], mybir.dt.float32, kind="ExternalInput").ap()
    t_ap = nc.dram_tensor("preds_t", [ROWS, C], mybir.dt.float32, kind="ExternalInput").ap()
    out_ap = nc.dram_tensor("partial", [P, NT], mybir.dt.float32, kind="ExternalOutput").ap()

    s3 = s_ap.rearrange("(n p) c -> n p c", p=P)
    t3 = t_ap.rearrange("(n p) c -> n p c", p=P)

    f32 = mybir.dt.float32
    s_tiles = [nc.alloc_sbuf_tensor(f"xent_s{i}", [P, C], f32) for i in range(NT)]
    t_tiles = [nc.alloc_sbuf_tensor(f"xent_t{i}", [P, C], f32) for i in range(NT)]
    log_tiles = [nc.alloc_sbuf_tensor(f"xent_log{i}", [P, C], f32) for i in range(NT)]
    acc = nc.alloc_sbuf_tensor("xent_acc", [P, NT], f32)
    dummy = nc.alloc_sbuf_tensor("xent_dummy", [P, 1], f32)

    dma_s = nc.alloc_semaphore("dma_s")
    dma_t = nc.alloc_semaphore("dma_t")
    act_done = nc.alloc_semaphore("act_done")
    dve_done = nc.alloc_semaphore("dve_done")
    out_done = nc.alloc_semaphore("out_done")

    with nc.Block() as block:

        @block.sync
        def _(sync):
            for i in range(NT):
                sync.dma_start(out=s_tiles[i].ap(), in_=s3[i]).then_inc(dma_s, 16)
            sync.wait_ge(dve_done, NT)
            sync.dma_start(out=out_ap, in_=acc.ap()).then_inc(out_done, 16)
            sync.wait_ge(out_done, 16)

        @block.gpsimd
        def _(gpsimd):
            for i in range(NT):
                gpsimd.dma_start(out=t_tiles[i].ap(), in_=t3[i]).then_inc(dma_t, 16)

        @block.scalar
        def _(scalar):
            for i in range(NT):
                scalar.wait_ge(dma_s, 16 * (i + 1))
                scalar.activation(
                    out=log_tiles[i].ap(),
                    in_=s_tiles[i].ap(),
                    func=mybir.ActivationFunctionType.Ln,
                ).then_inc(act_done, 1)

        @block.vector
        def _(vector):
            for i in range(NT):
                vector.wait_ge(act_done, i + 1)
                vector.wait_ge(dma_t, 16 * (i + 1))
                vector.scalar_tensor_tensor(
                    out=dummy.ap().broadcast_to([P, C]),
                    in0=log_tiles[i].ap(),
                    scalar=1.0,
                    in1=t_tiles[i].ap(),
                    op0=mybir.AluOpType.mult,
                    op1=mybir.AluOpType.mult,
                    accum_out=acc.ap()[:, i : i + 1],
                ).then_inc(dve_done, 1)

    nc.compile()
    _NC_CACHE["nc"] = nc
    return nc


def kernel(preds_s, preds_t):
    preds_s = np.ascontiguousarray(np.asarray(preds_s, dtype=np.float32))
    preds_t = np.ascontiguousarray(np.asarray(preds_t, dtype=np.float32))
    assert preds_s.shape == (B, C) and preds_t.shape == (B, C)

    nc = _build_nc()
    rs = preds_s.reshape(N_CORES, ROWS, C)
    rt = preds_t.reshape(N_CORES, ROWS, C)
    in_maps = [
        {"preds_s": np.ascontiguousarray(rs[k]), "preds_t": np.ascontiguousarray(rt[k])}
        for k in range(N_CORES)
    ]
    res = run_bass_kernel_spmd(nc, in_maps, core_ids=list(range(N_CORES)))
    total = 0.0
    for r in res.results:
        total += r["partial"].astype(np.float64).sum()
    return np.asarray(-total / B, dtype=np.float32)


# revision 9
# speedup vs baseline: 1.1180x; 1.0450x over previous
"""Distribution cross-entropy loss on 8 Trainium2 NeuronCores.

loss = -(1/B) * sum(preds_t * log(preds_s)),  preds_* : [4096, 1000] f32

Data-parallel: batch dim sharded 8 ways (512 rows/core). Per core, the
2x2MB shard is streamed through SBUF in [128,1000] tiles with the loads
spread over all three DMA-issuing engines (SP + ACT HWDGE rings, GpSimd
SWDGE ring) so descriptor generation and queue drain run in parallel.
ACT computes log, DVE does a fused multiply+row-sum (scalar_tensor_tensor
with a stride-0 dummy main output). Raw Bacc with manual semaphores (one
per DMA - a shared semaphore across DMAs on one queue is racy across the
16 SDMA engines). The Bass-init const barrier and Block-end barrier are
elided (no const APs are used; every DMA completion is semaphore-confirmed
before the consuming engine proceeds, so no trailing drain is needed).
Per-core output is a [128, NT] partial-sum tile; the final tiny reduction
over 8*128*NT values happens on the host in float64.
"""

import numpy as np

import concourse.bacc as bacc
import concourse.bass as bass
from concourse import mybir
from concourse.bass_utils import run_bass_kernel_spmd

N_CORES = 8
B, C = 4096, 1000
ROWS = B // N_CORES  # 512 rows per core
P = 128              # SBUF partitions
NT = ROWS // P       # 4 tiles per core

_NC_CACHE = {}


def _build_nc():
    if "nc" in _NC_CACHE:
        return _NC_CACHE["nc"]
    orig_barrier = bass.Bass.all_engine_barrier
    bass.Bass.all_engine_barrier = lambda self, *, sem_only=False: None
    try:
        nc = bacc.Bacc("TRN2", debug=False)
        f32 = mybir.dt.float32
        s_ap = nc.dram_tensor("preds_s", [ROWS, C], f32, kind="ExternalInput").ap()
        t_ap = nc.dram_tensor("preds_t", [ROWS, C], f32, kind="ExternalInput").ap()
        out_ap = nc.dram_tensor("partial", [P, NT], f32, kind="ExternalOutput").ap()

        s3 = s_ap.rearrange("(n p) c -> n p c", p=P)
        t3 = t_ap.rearrange("(n p) c -> n p c", p=P)

        s_tiles = [nc.alloc_sbuf_tensor(f"xent_s{i}", [P, C], f32) for i in range(NT)]
        t_tiles = [nc.alloc_sbuf_tensor(f"xent_t{i}", [P, C], f32) for i in range(NT)]
        log_tiles = [nc.alloc_sbuf_tensor(f"xent_log{i}", [P, C], f32) for i in range(NT)]
        acc = nc.alloc_sbuf_tensor("xent_acc", [P, NT], f32)
        dummy = nc.alloc_sbuf_tensor("xent_dummy", [P, 1], f32)
        bias = nc.alloc_sbuf_tensor("xent_bias", [P, 1], f32)

        sem_s = [nc.alloc_semaphore(f"sem_s{i}") for i in range(NT)]
        sem_t = [nc.alloc_semaphore(f"sem_t{i}") for i in range(NT)]
        act_done = nc.alloc_semaphore("act_done")
        dve_done = nc.alloc_semaphore("dve_done")
        out_done = nc.alloc_semaphore("out_done")
        bias_done = nc.alloc_semaphore("bias_done")

        with nc.Block() as block:

            @block.sync
            def _(sync):
                sync.dma_start(out=s_tiles[0].ap(), in_=s3[0]).then_inc(sem_s[0], 16)
                sync.dma_start(out=s_tiles[1].ap(), in_=s3[1]).then_inc(sem_s[1], 16)
                sync.wait_ge(dve_done, NT)
                sync.dma_start(out=out_ap, in_=acc.ap()).then_inc(out_done, 16)
                sync.wait_ge(out_done, 16)

            @block.gpsimd
            def _(gpsimd):
                for i in range(NT):
                    gpsimd.dma_start(out=t_tiles[i].ap(), in_=t3[i]).then_inc(sem_t[i], 16)

            @block.scalar
            def _(scalar):
                scalar.dma_start(out=s_tiles[2].ap(), in_=s3[2]).then_inc(sem_s[2], 16)
                scalar.dma_start(out=s_tiles[3].ap(), in_=s3[3]).then_inc(sem_s[3], 16)
                scalar.wait_ge(bias_done, 1)
                for i in range(NT):
                    scalar.wait_ge(sem_s[i], 16)
                    scalar.activation(
                        out=log_tiles[i].ap(),
                        in_=s_tiles[i].ap(),
                        func=mybir.ActivationFunctionType.Ln,
                        bias=bias.ap(),
                    ).then_inc(act_done, 1)

            @block.vector
            def _(vector):
                vector.memset(bias.ap(), 0.0).then_inc(bias_done, 1)
                for i in range(NT):
                    vector.wait_ge(act_done, i + 1)
                    vector.wait_ge(sem_t[i], 16)
                    vector.scalar_tensor_tensor(
                        out=dummy.ap().broadcast_to([P, C]),
                        in0=log_tiles[i].ap(),
                        scalar=1.0,
                        in1=t_tiles[i].ap(),
                        op0=mybir.AluOpType.mult,
                        op1=mybir.AluOpType.mult,
                        accum_out=acc.ap()[:, i : i + 1],
                    ).then_inc(dve_done, 1)

        nc.compile()
    finally:
        bass.Bass.all_engine_barrier = orig_barrier
    _NC_CACHE["nc"] = nc
    return nc


def kernel(preds_s, preds_t):
    preds_s = np.ascontiguousarray(np.asarray(preds_s, dtype=np.float32))
    preds_t = np.ascontiguousarray(np.asarray(preds_t, dtype=np.float32))
    assert preds_s.shape == (B, C) and preds_t.shape == (B, C)

    nc = _build_nc()
    rs = preds_s.reshape(N_CORES, ROWS, C)
    rt = preds_t.reshape(N_CORES, ROWS, C)
    in_maps = [
        {"preds_s": np.ascontiguousarray(rs[k]), "preds_t": np.ascontiguousarray(rt[k])}
        for k in range(N_CORES)
    ]
    res = run_bass_kernel_spmd(nc, in_maps, core_ids=list(range(N_CORES)))
    total = 0.0
    for r in res.results:
        total += r["partial"].astype(np.float64).sum()
    return np.asarray(-total / B, dtype=np.float32)


# revision 12
# speedup vs baseline: 1.2325x; 1.1024x over previous
"""Distribution cross-entropy loss on 8 Trainium2 NeuronCores.

loss = -(1/B) * sum(preds_t * log(preds_s)),  preds_* : [4096, 1000] f32

Data-parallel: batch dim sharded 8 ways (512 rows/core). Per core, the
2x2MB shard is streamed through SBUF in [128,1000] tiles with the loads
spread over all three DMA-issuing engines (SP + ACT HWDGE rings, GpSimd
SWDGE ring) so descriptor generation and queue drain run in parallel.
ACT computes log, DVE does a fused multiply+row-sum (scalar_tensor_tensor
with a stride-0 dummy main output). Raw Bacc with manual semaphores (one
per DMA - a shared semaphore across DMAs on one queue is racy across the
16 SDMA engines). The Bass-init const barrier and Block-end barrier are
elided (no const APs are used; every DMA completion is semaphore-confirmed
before the consuming engine proceeds, so no trailing drain is needed).
Per-core output is a [128, NT] partial-sum tile; the final tiny reduction
over 8*128*NT values happens on the host in float64.
"""

import numpy as np

import concourse.bacc as bacc
import concourse.bass as bass
from concourse import mybir
from concourse.bass_utils import run_bass_kernel_spmd

N_CORES = 8
B, C = 4096, 1000
ROWS = B // N_CORES  # 512 rows per core
P = 128              # SBUF partitions
NT = ROWS // P       # 4 tiles per core

_NC_CACHE = {}


def _build_nc():
    if "nc" in _NC_CACHE:
        return _NC_CACHE["nc"]
    orig_barrier = bass.Bass.all_engine_barrier
    bass.Bass.all_engine_barrier = lambda self, *, sem_only=False: None
    try:
        nc = bacc.Bacc("TRN2", debug=False)
        f32 = mybir.dt.float32
        s_ap = nc.dram_tensor("preds_s", [ROWS, C], f32, kind="ExternalInput").ap()
        t_ap = nc.dram_tensor("preds_t", [ROWS, C], f32, kind="ExternalInput").ap()
        out_ap = nc.dram_tensor("partial", [P, NT], f32, kind="ExternalOutput").ap()

        s3 = s_ap.rearrange("(n p) c -> n p c", p=P)
        t3 = t_ap.rearrange("(n p) c -> n p c", p=P)

        s_tiles = [nc.alloc_sbuf_tensor(f"xent_s{i}", [P, C], f32) for i in range(NT)]
        t_tiles = [nc.alloc_sbuf_tensor(f"xent_t{i}", [P, C], f32) for i in range(NT)]
        log_tiles = [nc.alloc_sbuf_tensor(f"xent_log{i}", [P, C], f32) for i in range(NT)]
        acc = nc.alloc_sbuf_tensor("xent_acc", [P, NT], f32)
        dummy = nc.alloc_sbuf_tensor("xent_dummy", [P, 1], f32)
        bias = nc.alloc_sbuf_tensor("xent_bias", [P, 1], f32)

        sem_s = [nc.alloc_semaphore(f"sem_s{i}") for i in range(NT)]
        sem_t = [nc.alloc_semaphore(f"sem_t{i}") for i in range(NT)]
        act_done = nc.alloc_semaphore("act_done")
        dve_done = nc.alloc_semaphore("dve_done")
        out_done = nc.alloc_semaphore("out_done")
        bias_done = nc.alloc_semaphore("bias_done")

        with nc.Block() as block:

            @block.sync
            def _(sync):
                # Single HWDGE queue, interleaved s/t: FIFO drain gives
                # ordered completions so compute pipelines behind the stream.
                for i in range(NT):
                    sync.dma_start(out=s_tiles[i].ap(), in_=s3[i]).then_inc(sem_s[i], 16)
                    sync.dma_start(out=t_tiles[i].ap(), in_=t3[i]).then_inc(sem_t[i], 16)
                sync.wait_ge(dve_done, NT)
                sync.dma_start(out=out_ap, in_=acc.ap()).then_inc(out_done, 16)
                sync.wait_ge(out_done, 16)

            @block.scalar
            def _(scalar):
                scalar.wait_ge(bias_done, 1)
                for i in range(NT):
                    scalar.wait_ge(sem_s[i], 16)
                    scalar.activation(
                        out=log_tiles[i].ap(),
                        in_=s_tiles[i].ap(),
                        func=mybir.ActivationFunctionType.Ln,
                        bias=bias.ap(),
                    ).then_inc(act_done, 1)

            @block.vector
            def _(vector):
                vector.memset(bias.ap(), 0.0).then_inc(bias_done, 1)
                for i in range(NT):
                    vector.wait_ge(act_done, i + 1)
                    vector.wait_ge(sem_t[i], 16)
                    vector.scalar_tensor_tensor(
                        out=dummy.ap().broadcast_to([P, C]),
                        in0=log_tiles[i].ap(),
                        scalar=1.0,
                        in1=t_tiles[i].ap(),
                        op0=mybir.AluOpType.mult,
                        op1=mybir.AluOpType.mult,
                        accum_out=acc.ap()[:, i : i + 1],
                    ).then_inc(dve_done, 1)

        nc.compile()
        # insert_act_table_loads places one LoadActFuncSet at the top of the
        # ACT block and a redundant second one right before the first
        # Activation (after the sem_s0 wait - i.e. on the critical path,
        # ~1.3us). The CFG is linear, so the first load suffices.
        for blk in nc.m.functions[0].blocks:
            seen_load = False
            for inst in list(blk.instructions):
                if isinstance(inst, mybir.InstLoadActFuncSet):
                    if seen_load:
                        blk.instructions.remove(inst)
                    seen_load = True
    finally:
        bass.Bass.all_engine_barrier = orig_barrier
    _NC_CACHE["nc"] = nc
    return nc


def kernel(preds_s, preds_t):
    preds_s = np.ascontiguousarray(np.asarray(preds_s, dtype=np.float32))
    preds_t = np.ascontiguousarray(np.asarray(preds_t, dtype=np.float32))
    assert preds_s.shape == (B, C) and preds_t.shape == (B, C)

    nc = _build_nc()
    rs = preds_s.reshape(N_CORES, ROWS, C)
    rt = preds_t.reshape(N_CORES, ROWS, C)
    in_maps = [
        {"preds_s": np.ascontiguousarray(rs[k]), "preds_t": np.ascontiguousarray(rt[k])}
        for k in range(N_CORES)
    ]
    res = run_bass_kernel_spmd(nc, in_maps, core_ids=list(range(N_CORES)))
    total = 0.0
    for r in res.results:
        total += r["partial"].astype(np.float64).sum()
    return np.asarray(-total / B, dtype=np.float32)
